# revision 36
# baseline (speedup 1.0000x reference)
"""MetaPathAgg Trainium2 kernel (8 NeuronCores, SPMD) — v4.

out[e] = LT_table[vote_lt[e]] + BV_table[vote_bv[e]]
  LT_table = h_lt @ W0 + (Mmem_norm @ h_comm) @ W3
             + (mean_don + mean_lob) @ W4 + b_fuse          (LT-sharded)
  BV_table[v] = mean_pv[v] @ W1 + (B_aug_v @ [h_comm@W2; h_topic@W5])[v]
                                                            (BV-sharded)

v4 (~0.57 ms) vs the 1.21 ms v3 baseline (trace-driven):
  * ALL segment sums (pv, don, lob) are dense window-compacted fp8/bf16
    matmuls over host-packed unique-source slabs — no SWDGE gathers and
    no gpsimd calls outside the final pass.  Slabs are host-swizzled to
    [128, X] partition-major so every stream is a fat contiguous DMA.
  * reciprocal normalization baked into A (pv) or applied by one DVE
    multiply per window flush (don/lob) — kills the 541 us of 1.9 us
    scalar.activation calls the baseline spent on scaling.
  * final pass: flat bv-sorted 128-edge slots (no window padding); BV
    side = compact per-slot one-hot fp8 matmul blocks (only windows any
    core actually touches) from SBUF-resident bvsb; LT side = one
    [P,1]-offset indirect row gather per slot.  A local region (edges
    whose LT row is core-owned) gathers from ltb_dram during the
    AllGather; the rest from ltfull_dram after it.
  * output written bf16 in [128, NS*D] partition-major swizzle; host
    unswizzles.

HW facts probed on this runtime (see probe_*.py):
  * multi-column indirect offset APs are SILENTLY wrong on HW: only
    offset column 0 is honored and extra out columns receive rows
    idx[p]+j (consecutive-row semantics).  Only [P,1] offsets are safe.
  * InstDMAGatherAnt (gpsimd.dma_gather) hard-crashes the device
    (NRT_EXEC_UNIT_UNRECOVERABLE); InstAPGather runs but the Q7 DSP copy
    rate (~255 us for 9.4k f32 columns) makes it useless here.
  * SWDGE indirect gathers cost ~1.4 us per 128-row call end to end
    (994 ns ucode desc-gen + ~0.3 us pacing) — the final-pass floor is
    NS * 1.4 us; the kernel overlaps ~1/8 of it with the collective.
"""

import os
import sys
import types

import numpy as np
import ml_dtypes

sys.path.insert(0, "/opt/trn_rl_repo")


def _ensure_ntff_hook():
    """Install antenv.axon_hooks if the image lacks it (trace=True path)."""
    try:
        from antenv.axon_hooks import get_axon_ntff_profile_hook  # noqa: F401
        return
    except ImportError:
        pass
    try:
        import antenv
        from trn_agent_boot.trn_boot import _ntff_profile_via_ctypes
        m = types.ModuleType("antenv.axon_hooks")
        holder = [None]
        m.set_axon_ntff_profile_hook = lambda h: holder.__setitem__(0, h)
        m.get_axon_ntff_profile_hook = lambda: holder[0]
        sys.modules["antenv.axon_hooks"] = m
        antenv.axon_hooks = m
        m.set_axon_ntff_profile_hook(
            _ntff_profile_via_ctypes("/opt/axon/libaxon_pjrt.so"))
    except Exception:
        pass


import concourse.bass as bass  # noqa: E402
import concourse.bacc as bacc  # noqa: E402
import concourse.mybir as mybir  # noqa: E402
import concourse.tile as tile  # noqa: E402

CORES = 8
P = 128
D = 128

F32 = mybir.dt.float32
BF16 = mybir.dt.bfloat16
F8 = mybir.dt.float8e4
I32 = mybir.dt.int32

BF = ml_dtypes.bfloat16
F8NP = ml_dtypes.float8_e4m3

_LAST_EXEC_NS = None
_LAST_RES = None


def _ceil(a, b):
    return (a + b - 1) // b


# ---------------------------------------------------------------------------
# host-side packing
# ---------------------------------------------------------------------------

def _pack_dense(src, dst_owner, dst_local, nloc, W, h_tab, h_np_dtype,
                bake_recip=None):
    """Window-compacted dense segment-sum slabs, uniform across cores.

    acc[d, loc] = sum_r h[r, d] * A[r, loc] per W-wide window of the
    local dst range.  Unique src rows per (core, window), padded to a
    block count uniform across cores per window.

    Returns per-core swizzled slabs:
      h_sb [CORES, 128, S*128] h_np_dtype   (h_sb[c, p, s*128+d] = h[blockrow p])
      a_sb [CORES, 128, S*W]   fp8          (a_sb[c, p, s*W+j])
      wos  [S] window of each slot
    If bake_recip is given (per-global-dst reciprocal), A holds
    count * recip; else raw counts.
    """
    nwin = nloc // W
    uniq = {}
    nblk_w = np.zeros(nwin, np.int64)
    for c in range(CORES):
        m = dst_owner == c
        s_c, l_c = src[m], dst_local[m]
        v_c = (bake_recip[m] if bake_recip is not None
               else np.ones(len(s_c), np.float32))
        for w in range(nwin):
            mw = (l_c // W) == w
            u = np.unique(s_c[mw])
            uniq[(c, w)] = (u, s_c[mw], l_c[mw], v_c[mw])
            nblk_w[w] = max(nblk_w[w], _ceil(len(u), P))
    slot_base = np.concatenate([[0], np.cumsum(nblk_w)]).astype(np.int64)
    S = int(slot_base[-1])
    wos = np.repeat(np.arange(nwin), nblk_w)
    h_sb = np.zeros((CORES, P, S * P), h_np_dtype)
    a_sb = np.zeros((CORES, P, S * W), F8NP)
    for c in range(CORES):
        for w in range(nwin):
            u, s_cw, l_cw, vals = uniq[(c, w)]
            n = len(u)
            if n == 0:
                continue
            inv = np.searchsorted(u, s_cw)
            a = np.zeros((nblk_w[w] * P, W), np.float32)
            np.add.at(a, (inv, l_cw % W), vals)
            hrows = np.zeros((nblk_w[w] * P, P), np.float32)
            hrows[:n] = h_tab[u]
            s0 = slot_base[w]
            for b in range(int(nblk_w[w])):
                s = s0 + b
                h_sb[c, :, s * P:(s + 1) * P] = \
                    hrows[b * P:(b + 1) * P].astype(h_np_dtype)
                a_sb[c, :, s * W:(s + 1) * W] = \
                    a[b * P:(b + 1) * P].astype(F8NP)
    return h_sb, a_sb, wos, S


def _prep(inputs):
    h_bv = np.asarray(inputs["h_bv"], np.float32)
    h_lt = np.asarray(inputs["h_lt"], np.float32)
    h_comm = np.asarray(inputs["h_comm"], np.float32)
    h_donor = np.asarray(inputs["h_donor"], np.float32)
    h_lobby = np.asarray(inputs["h_lobby"], np.float32)
    h_topic = np.asarray(inputs["h_topic"], np.float32)
    W_fuse = np.asarray(inputs["W_fuse"], np.float32)
    b_fuse = np.asarray(inputs["b_fuse"], np.float32)
    vote_lt = np.asarray(inputs["vote_lt"]).astype(np.int64)
    vote_bv = np.asarray(inputs["vote_bv"]).astype(np.int64)
    bv2bill = np.asarray(inputs["bv2bill"]).astype(np.int64)
    topic_ix = np.asarray(inputs["topic_ix"]).astype(np.int64)
    pv_src = np.asarray(inputs["pv_src"]).astype(np.int64)
    pv_dst = np.asarray(inputs["pv_dst"]).astype(np.int64)
    r_src = np.asarray(inputs["r_src"]).astype(np.int64)
    r_dst = np.asarray(inputs["r_dst"]).astype(np.int64)
    m_src = np.asarray(inputs["m_src"]).astype(np.int64)
    m_dst = np.asarray(inputs["m_dst"]).astype(np.int64)
    don_src = np.asarray(inputs["don_src"]).astype(np.int64)
    don_dst = np.asarray(inputs["don_dst"]).astype(np.int64)
    lob_src = np.asarray(inputs["lob_src"]).astype(np.int64)
    lob_dst = np.asarray(inputs["lob_dst"]).astype(np.int64)

    NBV = h_bv.shape[0]
    NLT = h_lt.shape[0]
    NB = np.asarray(inputs["h_bill"]).shape[0]
    NT = h_topic.shape[0]
    NCM = h_comm.shape[0]
    E = vote_lt.shape[0]
    assert NT <= P and NCM <= 2 * P

    # ---- sharding --------------------------------------------------------
    LTSH = _ceil(NLT, CORES)               # 625
    LLOC = _ceil(LTSH, P) * P              # 640
    NWL = LLOC // P                        # 5
    lt_owner = np.arange(NLT) // LTSH
    lt_local = np.arange(NLT) - lt_owner * LTSH

    VSH = _ceil(NBV, CORES)                # 12500
    VLOC = _ceil(VSH, P) * P               # 12544
    NWV = VLOC // P                        # 98
    v_owner = np.arange(NBV) // VSH
    v_local = np.arange(NBV) - v_owner * VSH

    # ---- dense segment-sum slabs ----------------------------------------
    WD = 32
    hdon, adon, wos_don, S_don = _pack_dense(
        don_src, lt_owner[don_dst], lt_local[don_dst], LLOC, WD, h_donor,
        F8NP)
    hlob, alob, wos_lob, S_lob = _pack_dense(
        lob_src, lt_owner[lob_dst], lt_local[lob_dst], LLOC, WD, h_lobby,
        F8NP)
    cnt_pv = np.bincount(pv_dst, minlength=NBV).astype(np.float32)
    recip_pv = 1.0 / np.maximum(cnt_pv, 1.0)
    hpv, apv, wos_pv, S_pv = _pack_dense(
        pv_src, v_owner[pv_dst], v_local[pv_dst], VLOC, P, h_bv, BF,
        bake_recip=recip_pv[pv_dst])

    # per-dst reciprocal slabs for don/lob (replicated over partitions, f32)
    def lt_recip_rep(dst):
        cnts = np.bincount(dst, minlength=NLT)
        r = np.ones((CORES, LLOC), np.float32)
        for c in range(CORES):
            lo = c * LTSH
            hi = min(lo + LTSH, NLT)
            r[c, :hi - lo] = 1.0 / np.maximum(cnts[lo:hi], 1)
        return np.repeat(r[:, None, :], P, axis=1).astype(BF)

    r_don = lt_recip_rep(don_dst)
    r_lob = lt_recip_rep(lob_dst)

    # ---- host folds: B_aug (read+topic to version rows), Mmem -----------
    nv = np.bincount(bv2bill, minlength=NB).astype(np.float64)
    cnt_rd = np.bincount(r_src, minlength=NBV).astype(np.float64)
    b_of_r = bv2bill[r_src]
    wgt = 1.0 / (np.maximum(cnt_rd[r_src], 1.0) * np.maximum(nv[b_of_r], 1.0))
    B_bill = np.zeros((NB, 3 * P), np.float32)
    np.add.at(B_bill, (b_of_r, r_dst), wgt.astype(np.float32))
    B_bill[np.arange(NB), 2 * P + topic_ix] = 1.0

    cnt_mem = np.bincount(m_src, minlength=NLT).astype(np.float64)
    Mmem = np.zeros((NLT, 2 * P), np.float32)
    np.add.at(Mmem, (m_src, m_dst),
              (1.0 / np.maximum(cnt_mem[m_src], 1.0)).astype(np.float32))

    BaT = np.zeros((CORES, P, NWV * 3 * P), BF)
    MmT = np.zeros((CORES, P, NWL * 2 * P), BF)
    for c in range(CORES):
        vlo = c * VSH
        vhi = min(vlo + VSH, NBV)
        Bv = np.zeros((VLOC, 3 * P), np.float32)
        Bv[: vhi - vlo] = B_bill[bv2bill[vlo:vhi]]
        t = Bv.reshape(NWV, P, 3, P)                  # [w, j, k, p]
        BaT[c] = t.transpose(3, 0, 2, 1).reshape(P, NWV * 3 * P).astype(BF)
        llo = c * LTSH
        lhi = min(llo + LTSH, NLT)
        Mv = np.zeros((LLOC, 2 * P), np.float32)
        Mv[: lhi - llo] = Mmem[llo:lhi]
        t2 = Mv.reshape(NWL, P, 2, P)
        MmT[c] = t2.transpose(3, 0, 2, 1).reshape(P, NWL * 2 * P).astype(BF)

    # ---- vote edges: flat bv-sorted slots -------------------------------
    # Region L (slots [0, NSL)): edges whose LT row is locally owned —
    # gathered from ltb_dram before the AllGather completes.  Region R:
    # the rest, gathered from ltfull_dram.  Both regions bv-sorted.
    ev_owner = v_owner[vote_bv]
    ev_local = v_local[vote_bv]
    lt_gidx = lt_owner * LLOC + lt_local
    core_local = []
    for c in range(CORES):
        ids = np.where(ev_owner == c)[0]
        isloc = lt_owner[vote_lt[ids]] == c
        loc = ids[isloc]
        core_local.append(loc[np.argsort(ev_local[loc], kind="stable")])
    NSL = min(len(x) for x in core_local) // P
    lwin = np.zeros(max(NSL, 1), np.int64)
    core_edges = []
    for c in range(CORES):
        ids = np.where(ev_owner == c)[0]
        loc = core_local[c][:NSL * P]
        rest = np.setdiff1d(ids, loc, assume_unique=True)
        rest = rest[np.argsort(ev_local[rest], kind="stable")]
        core_edges.append(np.concatenate([loc, rest]))
    NE = max(len(x) for x in core_edges)
    NS = _ceil(NE, P)
    NEP = NS * P

    # window coverage envelope: slot s covers bvsb windows
    # [wbase[s], wbase[s]+K)
    wfirst = np.full((CORES, NS), 10 ** 9, np.int64)
    wlast = np.zeros((CORES, NS), np.int64)
    for c in range(CORES):
        ids = core_edges[c]
        w = ev_local[ids] // P
        wpad = np.concatenate([w, np.repeat(w[-1], NEP - len(ids))])
        ws = wpad.reshape(NS, P)
        wfirst[c] = ws.min(axis=1)
        wlast[c] = ws.max(axis=1)
    wbase = wfirst.min(axis=0)
    K = int((wlast.max(axis=0) - wbase + 1).max())

    # per-slot used-k blocks, compacted: slot s owns O2 columns
    # [o2col[s], o2col[s+1]) * P; block i of slot s selects window wlist[s][i]
    kused = np.zeros((NS, K), bool)
    for c in range(CORES):
        ids = core_edges[c]
        n = len(ids)
        s = np.arange(n) // P
        k = ev_local[ids] // P - wbase[s]
        kused[s, k] = True
    ks_of = [np.nonzero(kused[s])[0] for s in range(NS)]
    o2col = np.concatenate([[0], np.cumsum([len(x) for x in ks_of])])
    NBLK = int(o2col[-1])
    wlist = [[min(int(wbase[s]) + int(k), NWV - 1) for k in ks_of[s]]
             for s in range(NS)]
    kpos = np.full((NS, K), -1, np.int64)
    for s in range(NS):
        for i, k in enumerate(ks_of[s]):
            kpos[s, k] = o2col[s] + i

    vlt = np.zeros((CORES, P, NS), np.int32)
    O2 = np.zeros((CORES, P, NBLK * P), F8NP)
    for c in range(CORES):
        ids = core_edges[c]
        n = len(ids)
        r = np.arange(n)
        p = r % P
        s = r // P
        gid = lt_gidx[vote_lt[ids]]
        lid = lt_local[vote_lt[ids]]
        vlt[c, p, s] = np.where(s < NSL, lid, gid)
        w = ev_local[ids] // P
        k = w - wbase[s]
        blk = kpos[s, k]
        assert (blk >= 0).all()
        off = ev_local[ids] % P
        O2[c, off, blk * P + p] = 1.0

    # ---- per-core dense inputs ------------------------------------------
    hltT = np.zeros((CORES, P, LLOC), BF)
    for c in range(CORES):
        lo = c * LTSH
        hi = min(lo + LTSH, NLT)
        hltT[c, :, : hi - lo] = h_lt[lo:hi].T.astype(BF)
    hcomT = np.zeros((P, 2 * P), BF)
    hcomT[:, :NCM] = h_comm.T.astype(BF)
    htopT = np.zeros((P, P), BF)
    htopT[:, :NT] = h_topic.T.astype(BF)
    wfb = W_fuse.astype(BF)
    biasm = np.tile(b_fuse[None, :], (P, 1)).astype(np.float32)

    in_maps = []
    for c in range(CORES):
        in_maps.append({
            "hdon": hdon[c], "adon": adon[c],
            "hlob": hlob[c], "alob": alob[c],
            "hpv": hpv[c], "apv": apv[c],
            "r_don": r_don[c], "r_lob": r_lob[c],
            "hltT": hltT[c], "hcomT": hcomT, "htopT": htopT,
            "wfb": wfb, "biasm": biasm,
            "BaT": BaT[c], "MmT": MmT[c],
            "vlt": vlt[c], "O2": O2[c],
        })

    plan = dict(
        NBV=NBV, NLT=NLT, NB=NB, E=E,
        LLOC=LLOC, NWL=NWL, VLOC=VLOC, NWV=NWV, WD=WD,
        S_don=S_don, wos_don=wos_don,
        S_lob=S_lob, wos_lob=wos_lob,
        S_pv=S_pv, wos_pv=wos_pv,
        NS=NS, NEP=NEP, K=K, wbase=wbase, NSL=NSL, kused=kused,
        o2col=o2col, NBLK=NBLK, wlist=wlist, lwin=lwin,
        core_edges=core_edges,
    )
    return plan, in_maps


# ---------------------------------------------------------------------------
# device program
# ---------------------------------------------------------------------------

def _build(plan):
    LLOC, NWL, WD = plan["LLOC"], plan["NWL"], plan["WD"]
    VLOC, NWV = plan["VLOC"], plan["NWV"]
    S_don, wos_don = plan["S_don"], plan["wos_don"]
    S_lob, wos_lob = plan["S_lob"], plan["wos_lob"]
    S_pv, wos_pv = plan["S_pv"], plan["wos_pv"]
    NS, K, wbase = plan["NS"], plan["K"], plan["wbase"]
    NSL, kused = plan["NSL"], plan["kused"]
    o2col, NBLK, wlist = plan["o2col"], plan["NBLK"], plan["wlist"]
    lwin = plan["lwin"]
    NWD = LLOC // WD

    nc = bacc.Bacc("TRN2", target_bir_lowering=False, debug=False,
                   num_devices=CORES)

    def din(name, shape, dt=BF16):
        return nc.dram_tensor(name, list(shape), dt, kind="ExternalInput")

    t_hdon = din("hdon", (P, S_don * P), F8)
    t_adon = din("adon", (P, S_don * WD), F8)
    t_hlob = din("hlob", (P, S_lob * P), F8)
    t_alob = din("alob", (P, S_lob * WD), F8)
    t_hpv = din("hpv", (P, S_pv * P))
    t_apv = din("apv", (P, S_pv * P), F8)
    t_rdon = din("r_don", (P, LLOC))
    t_rlob = din("r_lob", (P, LLOC))
    t_hltT = din("hltT", (P, LLOC))
    t_hcomT = din("hcomT", (P, 2 * P))
    t_htopT = din("htopT", (P, P))
    t_wfb = din("wfb", (6 * D, D))
    t_bias = din("biasm", (P, P), F32)
    t_BaT = din("BaT", (P, NWV * 3 * P))
    t_MmT = din("MmT", (P, NWL * 2 * P))
    t_vlt = din("vlt", (P, NS), I32)
    t_O2 = din("O2", (P, NBLK * P), F8)
    t_out = nc.dram_tensor("out", [P, NS * D], BF16, kind="ExternalOutput")

    with tile.TileContext(nc) as tc:
        with (
            tc.tile_pool(name="persist", bufs=1) as pp,
            tc.tile_pool(name="hstr", bufs=3) as hstr,
            tc.tile_pool(name="astr", bufs=3) as astr,
            tc.tile_pool(name="bstr", bufs=2) as bstr,
            tc.tile_pool(name="ostr", bufs=2) as ostr,
            tc.tile_pool(name="gpool", bufs=44) as gpool,
            tc.tile_pool(name="otile", bufs=2) as otile,
            tc.tile_pool(name="accps", bufs=2, space="PSUM") as accps,
            tc.tile_pool(name="tabps", bufs=2, space="PSUM") as tabps,
            tc.tile_pool(name="finps", bufs=4, space="PSUM") as finps,
            tc.tile_pool(name="dram", bufs=1, space="DRAM") as dram,
        ):
            def load(t, shape, dt=BF16, name=None, eng=nc.scalar):
                sb = pp.tile(list(shape), dt, name=name or (t.name + "_sb"))
                eng.dma_start(out=sb[:], in_=t.ap())
                return sb

            bias_sb = load(t_bias, (P, P), F32)
            hltT_sb = load(t_hltT, (P, LLOC))
            hcomT_sb = load(t_hcomT, (P, 2 * P))
            htopT_sb = load(t_htopT, (P, P))
            MmT_sb = load(t_MmT, (P, NWL * 2 * P))
            rdon_sb = load(t_rdon, (P, LLOC))
            rlob_sb = load(t_rlob, (P, LLOC))
            vlt_sb = load(t_vlt, (P, NS), I32)
            w_sb = []
            for k in range(6):
                wsb = pp.tile([P, D], BF16, name=f"w{k}_sb")
                nc.scalar.dma_start(out=wsb[:],
                                    in_=t_wfb.ap()[k * D:(k + 1) * D, :])
                w_sb.append(wsb)

            # DRAM intermediates
            ltb_dram = dram.tile([LLOC, D], BF16, name="ltb_dram")
            ltfull_dram = dram.tile([CORES * LLOC, D], BF16,
                                    addr_space="Shared", name="ltfull_dram")

            # ---- HW = [h_comm@W2 ; h_topic@W5], CW3 = h_comm@W3 ---------
            def proj(lhsT_ap, w_t, name):
                ps = tabps.tile([P, 512], F32, tag="tps", name=f"ps_{name}")
                nc.tensor.matmul(out=ps[:, :P], lhsT=lhsT_ap, rhs=w_t[:],
                                 start=True, stop=True)
                sb = pp.tile([P, D], BF16, name=name)
                nc.vector.tensor_copy(out=sb[:], in_=ps[:, :P])
                return sb

            # ---- dense segment-sum emitter ------------------------------
            def emit_dense(h_t, a_t, S, wos, W, acc, recip_sb, h_dt,
                           rn, CH, eng, slo=0, shi=None):
                """acc[d, w*W:(w+1)*W] = (sum_slots h_slot^T A_slot) * recip

                [slo, shi) restricts to a window-aligned slot range so the
                don/lob streams can interleave per LT window."""
                if shi is None:
                    shi = S
                first = {}
                last = {}
                for s, w in enumerate(wos):
                    w = int(w)
                    if w not in first:
                        first[w] = s
                    last[w] = s
                ps = {}
                for s0 in range(slo, shi, CH):
                    ns = min(CH, shi - s0)
                    ht = hstr.tile([P, ns * P], h_dt, tag=f"h{rn}",
                                   name=f"h_{rn}{s0}")
                    eng.dma_start(out=ht[:],
                                  in_=h_t.ap()[:, s0 * P:(s0 + ns) * P])
                    at = astr.tile([P, ns * W], F8, tag=f"a{rn}",
                                   name=f"a_{rn}{s0}")
                    eng.dma_start(out=at[:],
                                  in_=a_t.ap()[:, s0 * W:(s0 + ns) * W])
                    for j in range(ns):
                        s = s0 + j
                        w = int(wos[s])
                        if w not in ps:
                            ps[w] = accps.tile([P, 512], F32, tag="acc",
                                               name=f"ps_{rn}{w}")
                        nc.tensor.matmul(
                            out=ps[w][:, :W],
                            lhsT=ht[:, j * P:(j + 1) * P],
                            rhs=at[:, j * W:(j + 1) * W],
                            start=(s == first[w]), stop=(s == last[w]))
                        if s == last[w]:
                            if recip_sb is not None:
                                nc.vector.tensor_mul(
                                    out=acc[:, w * W:(w + 1) * W],
                                    in0=ps[w][:, :W],
                                    in1=recip_sb[:, w * W:(w + 1) * W])
                            else:
                                nc.vector.tensor_copy(
                                    out=acc[:, w * W:(w + 1) * W],
                                    in_=ps[w][:, :W])
                            del ps[w]

            acc_don = pp.tile([P, LLOC], BF16, name="acc_don")
            acc_lob = pp.tile([P, LLOC], BF16, name="acc_lob")
            acc_pv = pp.tile([P, VLOC], BF16, name="acc_pv")

            emit_dense(t_hdon, t_adon, S_don, wos_don, WD, acc_don,
                       rdon_sb, F8, "don", 64, nc.sync)
            emit_dense(t_hlob, t_alob, S_lob, wos_lob, WD, acc_lob,
                       rlob_sb, F8, "lob", 48, nc.sync)

            HW = [proj(hcomT_sb[:, :P], w_sb[2], "hw0"),
                  proj(hcomT_sb[:, P:2 * P], w_sb[2], "hw1"),
                  proj(htopT_sb[:], w_sb[5], "hw2")]
            CW3 = [proj(hcomT_sb[:, :P], w_sb[3], "cw30"),
                   proj(hcomT_sb[:, P:2 * P], w_sb[3], "cw31")]


            # ---- LT table -> ltb_dram -> AllGather ----------------------
            ltb_sb = pp.tile([P, NWL * D], BF16, name="ltb_sb")
            for w in range(NWL):
                sl = slice(w * P, (w + 1) * P)
                ps = tabps.tile([P, 512], F32, tag="tps", name=f"plt_{w}")
                nc.tensor.matmul(out=ps[:, :P], lhsT=hltT_sb[:, sl],
                                 rhs=w_sb[0][:], start=True, stop=False)
                for k in range(2):
                    nc.tensor.matmul(
                        out=ps[:, :P],
                        lhsT=MmT_sb[:, (w * 2 + k) * P:(w * 2 + k + 1) * P],
                        rhs=CW3[k][:], start=False, stop=False)
                nc.tensor.matmul(out=ps[:, :P], lhsT=acc_don[:, sl],
                                 rhs=w_sb[4][:], start=False, stop=False)
                nc.tensor.matmul(out=ps[:, :P], lhsT=acc_lob[:, sl],
                                 rhs=w_sb[4][:], start=False, stop=True)
                nc.vector.tensor_add(out=ltb_sb[:, w * D:(w + 1) * D],
                                     in0=ps[:, :P], in1=bias_sb[:])
            nc.sync.dma_start(
                out=ltb_dram[:].rearrange("(w p) d -> p w d", p=P),
                in_=ltb_sb[:].rearrange("p (w d) -> p w d", d=D))
            nc.gpsimd.collective_compute(
                "AllGather", mybir.AluOpType.bypass,
                replica_groups=[list(range(CORES))],
                ins=[ltb_dram.opt()], outs=[ltfull_dram.opt()])

            # ---- pv dense + BV table ------------------------------------
            emit_dense(t_hpv, t_apv, S_pv, wos_pv, P, acc_pv,
                       None, BF16, "pv", 32, nc.scalar)
            bvsb = pp.tile([P, NWV * D], BF16, name="bvsb")
            BW = 12  # BaT windows per streamed tile
            for w0 in range(0, NWV, BW):
                nw = min(BW, NWV - w0)
                bat = bstr.tile([P, nw * 3 * P], BF16, tag="bat",
                                name=f"bat_{w0}")
                nc.scalar.dma_start(
                    out=bat[:],
                    in_=t_BaT.ap()[:, w0 * 3 * P:(w0 + nw) * 3 * P])
                for wi in range(nw):
                    w = w0 + wi
                    psb = tabps.tile([P, 512], F32, tag="tps",
                                     name=f"psb_{w}")
                    nc.tensor.matmul(
                        out=psb[:, :P], lhsT=acc_pv[:, w * P:(w + 1) * P],
                        rhs=w_sb[1][:], start=True, stop=False)
                    for k in range(3):
                        nc.tensor.matmul(
                            out=psb[:, :P],
                            lhsT=bat[:, (wi * 3 + k) * P:(wi * 3 + k + 1) * P],
                            rhs=HW[k][:], start=False, stop=(k == 2))
                    nc.vector.tensor_copy(out=bvsb[:, w * D:(w + 1) * D],
                                          in_=psb[:, :P])

            # ---- final edge pass ----------------------------------------
            OCH = 16      # slots per output write
            OBLK = 64     # O2 blocks per stream tile (slot-aligned chunks)
            ot = None
            o2 = None
            chunk_s0 = 0
            chunk_b0 = 0
            for s in range(NS):
                if o2 is None or o2col[s + 1] - chunk_b0 > OBLK:
                    # start a new O2 chunk covering slots [s, s1)
                    s1 = s
                    while s1 < NS and o2col[s1 + 1] - o2col[s] <= OBLK:
                        s1 += 1
                    chunk_s0, chunk_b0 = s, int(o2col[s])
                    nb = int(o2col[s1]) - chunk_b0
                    o2 = ostr.tile([P, nb * P], F8, tag="o2",
                                   name=f"o2_{s}")
                    nc.scalar.dma_start(
                        out=o2[:],
                        in_=t_O2.ap()[:, chunk_b0 * P:(chunk_b0 + nb) * P])
                if s % OCH == 0:
                    nch = min(OCH, NS - s)
                    ot = otile.tile([P, nch * D], BF16, tag="ot",
                                    name=f"ot_{s}")
                glt = gpool.tile([P, D], BF16, tag="g", name=f"glt_{s}")
                nc.gpsimd.indirect_dma_start(
                    out=glt[:], out_offset=None,
                    in_=(ltb_dram[:] if s < NSL
                         else ltfull_dram[:]),
                    in_offset=bass.IndirectOffsetOnAxis(
                        ap=vlt_sb[:, s:s + 1], axis=0))
                ps = finps.tile([P, 512], F32, tag="fps", name=f"pfin_{s}")
                nk = int(o2col[s + 1]) - int(o2col[s])
                for i in range(nk):
                    b = int(o2col[s]) - chunk_b0 + i
                    nc.tensor.matmul(
                        out=ps[:, :P],
                        lhsT=o2[:, b * P:(b + 1) * P],
                        rhs=bvsb[:, wlist[s][i] * D:(wlist[s][i] + 1) * D],
                        start=(i == 0), stop=(i == nk - 1))
                nc.vector.tensor_add(
                    out=ot[:, (s % OCH) * D:(s % OCH + 1) * D],
                    in0=ps[:, :P], in1=glt[:])
                if s % OCH == OCH - 1 or s == NS - 1:
                    c0 = (s // OCH) * OCH
                    nc.scalar.dma_start(
                        out=t_out.ap()[:, c0 * D:(s + 1) * D],
                        in_=ot[:, :(s + 1 - c0) * D])

    nc.compile()
    return nc


# ---------------------------------------------------------------------------
# entry point
# ---------------------------------------------------------------------------

def kernel(**inputs):
    global _LAST_EXEC_NS, _LAST_RES
    plan, in_maps = _prep(inputs)
    nc = _build(plan)

    from concourse import bass_utils
    trace = os.environ.get("BASSK_TRACE", "0") == "1"
    _ensure_ntff_hook()
    res = bass_utils.run_bass_kernel_spmd(
        nc, in_maps, core_ids=list(range(CORES)), trace=trace)
    _LAST_EXEC_NS = res.exec_time_ns
    _LAST_RES = res

    E = plan["E"]
    NS = plan["NS"]
    out = np.zeros((E, D), np.float32)
    for c in range(CORES):
        ids = plan["core_edges"][c]
        rows = np.asarray(res.results[c]["out"]).astype(np.float32)
        rows = rows.reshape(P, NS, D).transpose(1, 0, 2).reshape(NS * P, D)
        out[ids] = rows[:len(ids)]
    return out


# revision 37
# speedup vs baseline: 1.0050x; 1.0050x over previous
"""MetaPathAgg Trainium2 kernel (8 NeuronCores, SPMD) — v4.

out[e] = LT_table[vote_lt[e]] + BV_table[vote_bv[e]]
  LT_table = h_lt @ W0 + (Mmem_norm @ h_comm) @ W3
             + (mean_don + mean_lob) @ W4 + b_fuse          (LT-sharded)
  BV_table[v] = mean_pv[v] @ W1 + (B_aug_v @ [h_comm@W2; h_topic@W5])[v]
                                                            (BV-sharded)

v4 (~0.57 ms) vs the 1.21 ms v3 baseline (trace-driven):
  * ALL segment sums (pv, don, lob) are dense window-compacted fp8/bf16
    matmuls over host-packed unique-source slabs — no SWDGE gathers and
    no gpsimd calls outside the final pass.  Slabs are host-swizzled to
    [128, X] partition-major so every stream is a fat contiguous DMA.
  * reciprocal normalization baked into A (pv) or applied by one DVE
    multiply per window flush (don/lob) — kills the 541 us of 1.9 us
    scalar.activation calls the baseline spent on scaling.
  * final pass: flat bv-sorted 128-edge slots (no window padding); BV
    side = compact per-slot one-hot fp8 matmul blocks (only windows any
    core actually touches) from SBUF-resident bvsb; LT side = one
    [P,1]-offset indirect row gather per slot.  A local region (edges
    whose LT row is core-owned) gathers from ltb_dram during the
    AllGather; the rest from ltfull_dram after it.
  * output written bf16 in [128, NS*D] partition-major swizzle; host
    unswizzles.

HW facts probed on this runtime (see probe_*.py):
  * multi-column indirect offset APs are SILENTLY wrong on HW: only
    offset column 0 is honored and extra out columns receive rows
    idx[p]+j (consecutive-row semantics).  Only [P,1] offsets are safe.
  * InstDMAGatherAnt (gpsimd.dma_gather) hard-crashes the device
    (NRT_EXEC_UNIT_UNRECOVERABLE); InstAPGather runs but the Q7 DSP copy
    rate (~255 us for 9.4k f32 columns) makes it useless here.
  * SWDGE indirect gathers cost ~1.4 us per 128-row call end to end
    (994 ns ucode desc-gen + ~0.3 us pacing) — the final-pass floor is
    NS * 1.4 us; the kernel overlaps ~1/8 of it with the collective.
"""

import os
import sys
import types

import numpy as np
import ml_dtypes

sys.path.insert(0, "/opt/trn_rl_repo")


def _ensure_ntff_hook():
    """Install antenv.axon_hooks if the image lacks it (trace=True path)."""
    try:
        from antenv.axon_hooks import get_axon_ntff_profile_hook  # noqa: F401
        return
    except ImportError:
        pass
    try:
        import antenv
        from trn_agent_boot.trn_boot import _ntff_profile_via_ctypes
        m = types.ModuleType("antenv.axon_hooks")
        holder = [None]
        m.set_axon_ntff_profile_hook = lambda h: holder.__setitem__(0, h)
        m.get_axon_ntff_profile_hook = lambda: holder[0]
        sys.modules["antenv.axon_hooks"] = m
        antenv.axon_hooks = m
        m.set_axon_ntff_profile_hook(
            _ntff_profile_via_ctypes("/opt/axon/libaxon_pjrt.so"))
    except Exception:
        pass


import concourse.bass as bass  # noqa: E402
import concourse.bacc as bacc  # noqa: E402
import concourse.mybir as mybir  # noqa: E402
import concourse.tile as tile  # noqa: E402

CORES = 8
P = 128
D = 128

F32 = mybir.dt.float32
BF16 = mybir.dt.bfloat16
F8 = mybir.dt.float8e4
I32 = mybir.dt.int32

BF = ml_dtypes.bfloat16
F8NP = ml_dtypes.float8_e4m3

_LAST_EXEC_NS = None
_LAST_RES = None


def _ceil(a, b):
    return (a + b - 1) // b


# ---------------------------------------------------------------------------
# host-side packing
# ---------------------------------------------------------------------------

def _pack_dense(src, dst_owner, dst_local, nloc, W, h_tab, h_np_dtype,
                bake_recip=None):
    """Window-compacted dense segment-sum slabs, uniform across cores.

    acc[d, loc] = sum_r h[r, d] * A[r, loc] per W-wide window of the
    local dst range.  Unique src rows per (core, window), padded to a
    block count uniform across cores per window.

    Returns per-core swizzled slabs:
      h_sb [CORES, 128, S*128] h_np_dtype   (h_sb[c, p, s*128+d] = h[blockrow p])
      a_sb [CORES, 128, S*W]   fp8          (a_sb[c, p, s*W+j])
      wos  [S] window of each slot
    If bake_recip is given (per-global-dst reciprocal), A holds
    count * recip; else raw counts.
    """
    nwin = nloc // W
    uniq = {}
    nblk_w = np.zeros(nwin, np.int64)
    for c in range(CORES):
        m = dst_owner == c
        s_c, l_c = src[m], dst_local[m]
        v_c = (bake_recip[m] if bake_recip is not None
               else np.ones(len(s_c), np.float32))
        for w in range(nwin):
            mw = (l_c // W) == w
            u = np.unique(s_c[mw])
            uniq[(c, w)] = (u, s_c[mw], l_c[mw], v_c[mw])
            nblk_w[w] = max(nblk_w[w], _ceil(len(u), P))
    slot_base = np.concatenate([[0], np.cumsum(nblk_w)]).astype(np.int64)
    S = int(slot_base[-1])
    wos = np.repeat(np.arange(nwin), nblk_w)
    h_sb = np.zeros((CORES, P, S * P), h_np_dtype)
    a_sb = np.zeros((CORES, P, S * W), F8NP)
    for c in range(CORES):
        for w in range(nwin):
            u, s_cw, l_cw, vals = uniq[(c, w)]
            n = len(u)
            if n == 0:
                continue
            inv = np.searchsorted(u, s_cw)
            a = np.zeros((nblk_w[w] * P, W), np.float32)
            np.add.at(a, (inv, l_cw % W), vals)
            hrows = np.zeros((nblk_w[w] * P, P), np.float32)
            hrows[:n] = h_tab[u]
            s0 = slot_base[w]
            for b in range(int(nblk_w[w])):
                s = s0 + b
                h_sb[c, :, s * P:(s + 1) * P] = \
                    hrows[b * P:(b + 1) * P].astype(h_np_dtype)
                a_sb[c, :, s * W:(s + 1) * W] = \
                    a[b * P:(b + 1) * P].astype(F8NP)
    return h_sb, a_sb, wos, S


def _prep(inputs):
    h_bv = np.asarray(inputs["h_bv"], np.float32)
    h_lt = np.asarray(inputs["h_lt"], np.float32)
    h_comm = np.asarray(inputs["h_comm"], np.float32)
    h_donor = np.asarray(inputs["h_donor"], np.float32)
    h_lobby = np.asarray(inputs["h_lobby"], np.float32)
    h_topic = np.asarray(inputs["h_topic"], np.float32)
    W_fuse = np.asarray(inputs["W_fuse"], np.float32)
    b_fuse = np.asarray(inputs["b_fuse"], np.float32)
    vote_lt = np.asarray(inputs["vote_lt"]).astype(np.int64)
    vote_bv = np.asarray(inputs["vote_bv"]).astype(np.int64)
    bv2bill = np.asarray(inputs["bv2bill"]).astype(np.int64)
    topic_ix = np.asarray(inputs["topic_ix"]).astype(np.int64)
    pv_src = np.asarray(inputs["pv_src"]).astype(np.int64)
    pv_dst = np.asarray(inputs["pv_dst"]).astype(np.int64)
    r_src = np.asarray(inputs["r_src"]).astype(np.int64)
    r_dst = np.asarray(inputs["r_dst"]).astype(np.int64)
    m_src = np.asarray(inputs["m_src"]).astype(np.int64)
    m_dst = np.asarray(inputs["m_dst"]).astype(np.int64)
    don_src = np.asarray(inputs["don_src"]).astype(np.int64)
    don_dst = np.asarray(inputs["don_dst"]).astype(np.int64)
    lob_src = np.asarray(inputs["lob_src"]).astype(np.int64)
    lob_dst = np.asarray(inputs["lob_dst"]).astype(np.int64)

    NBV = h_bv.shape[0]
    NLT = h_lt.shape[0]
    NB = np.asarray(inputs["h_bill"]).shape[0]
    NT = h_topic.shape[0]
    NCM = h_comm.shape[0]
    E = vote_lt.shape[0]
    assert NT <= P and NCM <= 2 * P

    # ---- sharding --------------------------------------------------------
    LTSH = _ceil(NLT, CORES)               # 625
    LLOC = _ceil(LTSH, P) * P              # 640
    NWL = LLOC // P                        # 5
    lt_owner = np.arange(NLT) // LTSH
    lt_local = np.arange(NLT) - lt_owner * LTSH

    VSH = _ceil(NBV, CORES)                # 12500
    VLOC = _ceil(VSH, P) * P               # 12544
    NWV = VLOC // P                        # 98
    v_owner = np.arange(NBV) // VSH
    v_local = np.arange(NBV) - v_owner * VSH

    # ---- dense segment-sum slabs ----------------------------------------
    WD = 32
    hdon, adon, wos_don, S_don = _pack_dense(
        don_src, lt_owner[don_dst], lt_local[don_dst], LLOC, WD, h_donor,
        F8NP)
    hlob, alob, wos_lob, S_lob = _pack_dense(
        lob_src, lt_owner[lob_dst], lt_local[lob_dst], LLOC, WD, h_lobby,
        F8NP)
    cnt_pv = np.bincount(pv_dst, minlength=NBV).astype(np.float32)
    recip_pv = 1.0 / np.maximum(cnt_pv, 1.0)
    hpv, apv, wos_pv, S_pv = _pack_dense(
        pv_src, v_owner[pv_dst], v_local[pv_dst], VLOC, P, h_bv, BF,
        bake_recip=recip_pv[pv_dst])

    # per-dst reciprocal slabs for don/lob (replicated over partitions, f32)
    def lt_recip_rep(dst):
        cnts = np.bincount(dst, minlength=NLT)
        r = np.ones((CORES, LLOC), np.float32)
        for c in range(CORES):
            lo = c * LTSH
            hi = min(lo + LTSH, NLT)
            r[c, :hi - lo] = 1.0 / np.maximum(cnts[lo:hi], 1)
        return np.repeat(r[:, None, :], P, axis=1).astype(BF)

    r_don = lt_recip_rep(don_dst)
    r_lob = lt_recip_rep(lob_dst)

    # ---- host folds: B_aug (read+topic to version rows), Mmem -----------
    nv = np.bincount(bv2bill, minlength=NB).astype(np.float64)
    cnt_rd = np.bincount(r_src, minlength=NBV).astype(np.float64)
    b_of_r = bv2bill[r_src]
    wgt = 1.0 / (np.maximum(cnt_rd[r_src], 1.0) * np.maximum(nv[b_of_r], 1.0))
    B_bill = np.zeros((NB, 3 * P), np.float32)
    np.add.at(B_bill, (b_of_r, r_dst), wgt.astype(np.float32))
    B_bill[np.arange(NB), 2 * P + topic_ix] = 1.0

    cnt_mem = np.bincount(m_src, minlength=NLT).astype(np.float64)
    Mmem = np.zeros((NLT, 2 * P), np.float32)
    np.add.at(Mmem, (m_src, m_dst),
              (1.0 / np.maximum(cnt_mem[m_src], 1.0)).astype(np.float32))

    BaT = np.zeros((CORES, P, NWV * 3 * P), BF)
    MmT = np.zeros((CORES, P, NWL * 2 * P), BF)
    for c in range(CORES):
        vlo = c * VSH
        vhi = min(vlo + VSH, NBV)
        Bv = np.zeros((VLOC, 3 * P), np.float32)
        Bv[: vhi - vlo] = B_bill[bv2bill[vlo:vhi]]
        t = Bv.reshape(NWV, P, 3, P)                  # [w, j, k, p]
        BaT[c] = t.transpose(3, 0, 2, 1).reshape(P, NWV * 3 * P).astype(BF)
        llo = c * LTSH
        lhi = min(llo + LTSH, NLT)
        Mv = np.zeros((LLOC, 2 * P), np.float32)
        Mv[: lhi - llo] = Mmem[llo:lhi]
        t2 = Mv.reshape(NWL, P, 2, P)
        MmT[c] = t2.transpose(3, 0, 2, 1).reshape(P, NWL * 2 * P).astype(BF)

    # ---- vote edges: flat bv-sorted slots -------------------------------
    # Region L (slots [0, NSL)): edges whose LT row is locally owned —
    # gathered from ltb_dram before the AllGather completes.  Region R:
    # the rest, gathered from ltfull_dram.  Both regions bv-sorted.
    ev_owner = v_owner[vote_bv]
    ev_local = v_local[vote_bv]
    lt_gidx = lt_owner * LLOC + lt_local
    core_local = []
    for c in range(CORES):
        ids = np.where(ev_owner == c)[0]
        isloc = lt_owner[vote_lt[ids]] == c
        loc = ids[isloc]
        core_local.append(loc[np.argsort(ev_local[loc], kind="stable")])
    NSL = min(len(x) for x in core_local) // P
    lwin = np.zeros(max(NSL, 1), np.int64)
    core_edges = []
    for c in range(CORES):
        ids = np.where(ev_owner == c)[0]
        loc = core_local[c][:NSL * P]
        rest = np.setdiff1d(ids, loc, assume_unique=True)
        rest = rest[np.argsort(ev_local[rest], kind="stable")]
        core_edges.append(np.concatenate([loc, rest]))
    NE = max(len(x) for x in core_edges)
    NS = _ceil(NE, P)
    NEP = NS * P

    # window coverage envelope: slot s covers bvsb windows
    # [wbase[s], wbase[s]+K)
    wfirst = np.full((CORES, NS), 10 ** 9, np.int64)
    wlast = np.zeros((CORES, NS), np.int64)
    for c in range(CORES):
        ids = core_edges[c]
        w = ev_local[ids] // P
        wpad = np.concatenate([w, np.repeat(w[-1], NEP - len(ids))])
        ws = wpad.reshape(NS, P)
        wfirst[c] = ws.min(axis=1)
        wlast[c] = ws.max(axis=1)
    wbase = wfirst.min(axis=0)
    K = int((wlast.max(axis=0) - wbase + 1).max())

    # per-slot used-k blocks, compacted: slot s owns O2 columns
    # [o2col[s], o2col[s+1]) * P; block i of slot s selects window wlist[s][i]
    kused = np.zeros((NS, K), bool)
    for c in range(CORES):
        ids = core_edges[c]
        n = len(ids)
        s = np.arange(n) // P
        k = ev_local[ids] // P - wbase[s]
        kused[s, k] = True
    ks_of = [np.nonzero(kused[s])[0] for s in range(NS)]
    o2col = np.concatenate([[0], np.cumsum([len(x) for x in ks_of])])
    NBLK = int(o2col[-1])
    wlist = [[min(int(wbase[s]) + int(k), NWV - 1) for k in ks_of[s]]
             for s in range(NS)]
    kpos = np.full((NS, K), -1, np.int64)
    for s in range(NS):
        for i, k in enumerate(ks_of[s]):
            kpos[s, k] = o2col[s] + i

    vlt = np.zeros((CORES, P, NS), np.int32)
    O2 = np.zeros((CORES, P, NBLK * P), F8NP)
    for c in range(CORES):
        ids = core_edges[c]
        n = len(ids)
        r = np.arange(n)
        p = r % P
        s = r // P
        gid = lt_gidx[vote_lt[ids]]
        lid = lt_local[vote_lt[ids]]
        vlt[c, p, s] = np.where(s < NSL, lid, gid)
        w = ev_local[ids] // P
        k = w - wbase[s]
        blk = kpos[s, k]
        assert (blk >= 0).all()
        off = ev_local[ids] % P
        O2[c, off, blk * P + p] = 1.0

    # ---- per-core dense inputs ------------------------------------------
    hltT = np.zeros((CORES, P, LLOC), BF)
    for c in range(CORES):
        lo = c * LTSH
        hi = min(lo + LTSH, NLT)
        hltT[c, :, : hi - lo] = h_lt[lo:hi].T.astype(BF)
    hcomT = np.zeros((P, 2 * P), BF)
    hcomT[:, :NCM] = h_comm.T.astype(BF)
    htopT = np.zeros((P, P), BF)
    htopT[:, :NT] = h_topic.T.astype(BF)
    wfb = W_fuse.astype(BF)
    biasm = np.tile(b_fuse[None, :], (P, 1)).astype(np.float32)

    in_maps = []
    for c in range(CORES):
        in_maps.append({
            "hdon": hdon[c], "adon": adon[c],
            "hlob": hlob[c], "alob": alob[c],
            "hpv": hpv[c], "apv": apv[c],
            "r_don": r_don[c], "r_lob": r_lob[c],
            "hltT": hltT[c], "hcomT": hcomT, "htopT": htopT,
            "wfb": wfb, "biasm": biasm,
            "BaT": BaT[c], "MmT": MmT[c],
            "vlt": vlt[c], "O2": O2[c],
        })

    plan = dict(
        NBV=NBV, NLT=NLT, NB=NB, E=E,
        LLOC=LLOC, NWL=NWL, VLOC=VLOC, NWV=NWV, WD=WD,
        S_don=S_don, wos_don=wos_don,
        S_lob=S_lob, wos_lob=wos_lob,
        S_pv=S_pv, wos_pv=wos_pv,
        NS=NS, NEP=NEP, K=K, wbase=wbase, NSL=NSL, kused=kused,
        o2col=o2col, NBLK=NBLK, wlist=wlist, lwin=lwin,
        core_edges=core_edges,
    )
    return plan, in_maps


# ---------------------------------------------------------------------------
# device program
# ---------------------------------------------------------------------------

def _build(plan):
    LLOC, NWL, WD = plan["LLOC"], plan["NWL"], plan["WD"]
    VLOC, NWV = plan["VLOC"], plan["NWV"]
    S_don, wos_don = plan["S_don"], plan["wos_don"]
    S_lob, wos_lob = plan["S_lob"], plan["wos_lob"]
    S_pv, wos_pv = plan["S_pv"], plan["wos_pv"]
    NS, K, wbase = plan["NS"], plan["K"], plan["wbase"]
    NSL, kused = plan["NSL"], plan["kused"]
    o2col, NBLK, wlist = plan["o2col"], plan["NBLK"], plan["wlist"]
    lwin = plan["lwin"]
    NWD = LLOC // WD

    nc = bacc.Bacc("TRN2", target_bir_lowering=False, debug=False,
                   num_devices=CORES)

    def din(name, shape, dt=BF16):
        return nc.dram_tensor(name, list(shape), dt, kind="ExternalInput")

    t_hdon = din("hdon", (P, S_don * P), F8)
    t_adon = din("adon", (P, S_don * WD), F8)
    t_hlob = din("hlob", (P, S_lob * P), F8)
    t_alob = din("alob", (P, S_lob * WD), F8)
    t_hpv = din("hpv", (P, S_pv * P))
    t_apv = din("apv", (P, S_pv * P), F8)
    t_rdon = din("r_don", (P, LLOC))
    t_rlob = din("r_lob", (P, LLOC))
    t_hltT = din("hltT", (P, LLOC))
    t_hcomT = din("hcomT", (P, 2 * P))
    t_htopT = din("htopT", (P, P))
    t_wfb = din("wfb", (6 * D, D))
    t_bias = din("biasm", (P, P), F32)
    t_BaT = din("BaT", (P, NWV * 3 * P))
    t_MmT = din("MmT", (P, NWL * 2 * P))
    t_vlt = din("vlt", (P, NS), I32)
    t_O2 = din("O2", (P, NBLK * P), F8)
    t_out = nc.dram_tensor("out", [P, NS * D], BF16, kind="ExternalOutput")

    with tile.TileContext(nc) as tc:
        with (
            tc.tile_pool(name="persist", bufs=1) as pp,
            tc.tile_pool(name="hstr", bufs=3) as hstr,
            tc.tile_pool(name="astr", bufs=3) as astr,
            tc.tile_pool(name="bstr", bufs=2) as bstr,
            tc.tile_pool(name="ostr", bufs=2) as ostr,
            tc.tile_pool(name="gpool", bufs=44) as gpool,
            tc.tile_pool(name="otile", bufs=2) as otile,
            tc.tile_pool(name="accps", bufs=2, space="PSUM") as accps,
            tc.tile_pool(name="tabps", bufs=2, space="PSUM") as tabps,
            tc.tile_pool(name="finps", bufs=4, space="PSUM") as finps,
            tc.tile_pool(name="dram", bufs=1, space="DRAM") as dram,
        ):
            def load(t, shape, dt=BF16, name=None, eng=nc.scalar):
                sb = pp.tile(list(shape), dt, name=name or (t.name + "_sb"))
                eng.dma_start(out=sb[:], in_=t.ap())
                return sb

            bias_sb = load(t_bias, (P, P), F32)
            hltT_sb = load(t_hltT, (P, LLOC))
            hcomT_sb = load(t_hcomT, (P, 2 * P))
            htopT_sb = load(t_htopT, (P, P))
            MmT_sb = load(t_MmT, (P, NWL * 2 * P))
            rdon_sb = load(t_rdon, (P, LLOC))
            rlob_sb = load(t_rlob, (P, LLOC))
            vlt_sb = load(t_vlt, (P, NS), I32)
            w_sb = []
            for k in range(6):
                wsb = pp.tile([P, D], BF16, name=f"w{k}_sb")
                nc.scalar.dma_start(out=wsb[:],
                                    in_=t_wfb.ap()[k * D:(k + 1) * D, :])
                w_sb.append(wsb)

            # DRAM intermediates
            ltb_dram = dram.tile([LLOC, D], BF16, name="ltb_dram")
            ltfull_dram = dram.tile([CORES * LLOC, D], BF16,
                                    addr_space="Shared", name="ltfull_dram")

            # ---- HW = [h_comm@W2 ; h_topic@W5], CW3 = h_comm@W3 ---------
            def proj(lhsT_ap, w_t, name):
                ps = tabps.tile([P, 512], F32, tag="tps", name=f"ps_{name}")
                nc.tensor.matmul(out=ps[:, :P], lhsT=lhsT_ap, rhs=w_t[:],
                                 start=True, stop=True)
                sb = pp.tile([P, D], BF16, name=name)
                nc.vector.tensor_copy(out=sb[:], in_=ps[:, :P])
                return sb

            # ---- dense segment-sum emitter ------------------------------
            def emit_dense(h_t, a_t, S, wos, W, acc, recip_sb, h_dt,
                           rn, CH, eng, slo=0, shi=None):
                """acc[d, w*W:(w+1)*W] = (sum_slots h_slot^T A_slot) * recip

                [slo, shi) restricts to a window-aligned slot range so the
                don/lob streams can interleave per LT window."""
                if shi is None:
                    shi = S
                first = {}
                last = {}
                for s, w in enumerate(wos):
                    w = int(w)
                    if w not in first:
                        first[w] = s
                    last[w] = s
                ps = {}
                for s0 in range(slo, shi, CH):
                    ns = min(CH, shi - s0)
                    ht = hstr.tile([P, ns * P], h_dt, tag=f"h{rn}",
                                   name=f"h_{rn}{s0}")
                    eng.dma_start(out=ht[:],
                                  in_=h_t.ap()[:, s0 * P:(s0 + ns) * P])
                    at = astr.tile([P, ns * W], F8, tag=f"a{rn}",
                                   name=f"a_{rn}{s0}")
                    eng.dma_start(out=at[:],
                                  in_=a_t.ap()[:, s0 * W:(s0 + ns) * W])
                    for j in range(ns):
                        s = s0 + j
                        w = int(wos[s])
                        if w not in ps:
                            ps[w] = accps.tile([P, 512], F32, tag="acc",
                                               name=f"ps_{rn}{w}")
                        nc.tensor.matmul(
                            out=ps[w][:, :W],
                            lhsT=ht[:, j * P:(j + 1) * P],
                            rhs=at[:, j * W:(j + 1) * W],
                            start=(s == first[w]), stop=(s == last[w]))
                        if s == last[w]:
                            if recip_sb is not None:
                                nc.vector.tensor_mul(
                                    out=acc[:, w * W:(w + 1) * W],
                                    in0=ps[w][:, :W],
                                    in1=recip_sb[:, w * W:(w + 1) * W])
                            else:
                                nc.vector.tensor_copy(
                                    out=acc[:, w * W:(w + 1) * W],
                                    in_=ps[w][:, :W])
                            del ps[w]

            acc_don = pp.tile([P, LLOC], BF16, name="acc_don")
            acc_lob = pp.tile([P, LLOC], BF16, name="acc_lob")
            acc_pv = pp.tile([P, VLOC], BF16, name="acc_pv")

            emit_dense(t_hdon, t_adon, S_don, wos_don, WD, acc_don,
                       rdon_sb, F8, "don", 64, nc.sync)
            emit_dense(t_hlob, t_alob, S_lob, wos_lob, WD, acc_lob,
                       rlob_sb, F8, "lob", 48, nc.sync)

            HW = [proj(hcomT_sb[:, :P], w_sb[2], "hw0"),
                  proj(hcomT_sb[:, P:2 * P], w_sb[2], "hw1"),
                  proj(htopT_sb[:], w_sb[5], "hw2")]
            CW3 = [proj(hcomT_sb[:, :P], w_sb[3], "cw30"),
                   proj(hcomT_sb[:, P:2 * P], w_sb[3], "cw31")]


            # pv dense before the LT build so the BV-table build starts
            # early enough to stay ahead of the final chain's adds
            emit_dense(t_hpv, t_apv, S_pv, wos_pv, P, acc_pv,
                       None, BF16, "pv", 32, nc.scalar)

            # ---- LT table -> ltb_dram -> AllGather ----------------------
            ltb_sb = pp.tile([P, NWL * D], BF16, name="ltb_sb")
            for w in range(NWL):
                sl = slice(w * P, (w + 1) * P)
                ps = tabps.tile([P, 512], F32, tag="tps", name=f"plt_{w}")
                nc.tensor.matmul(out=ps[:, :P], lhsT=hltT_sb[:, sl],
                                 rhs=w_sb[0][:], start=True, stop=False)
                for k in range(2):
                    nc.tensor.matmul(
                        out=ps[:, :P],
                        lhsT=MmT_sb[:, (w * 2 + k) * P:(w * 2 + k + 1) * P],
                        rhs=CW3[k][:], start=False, stop=False)
                nc.tensor.matmul(out=ps[:, :P], lhsT=acc_don[:, sl],
                                 rhs=w_sb[4][:], start=False, stop=False)
                nc.tensor.matmul(out=ps[:, :P], lhsT=acc_lob[:, sl],
                                 rhs=w_sb[4][:], start=False, stop=True)
                nc.vector.tensor_add(out=ltb_sb[:, w * D:(w + 1) * D],
                                     in0=ps[:, :P], in1=bias_sb[:])
            nc.sync.dma_start(
                out=ltb_dram[:].rearrange("(w p) d -> p w d", p=P),
                in_=ltb_sb[:].rearrange("p (w d) -> p w d", d=D))
            nc.gpsimd.collective_compute(
                "AllGather", mybir.AluOpType.bypass,
                replica_groups=[list(range(CORES))],
                ins=[ltb_dram.opt()], outs=[ltfull_dram.opt()])

            # ---- BV table (pv dense emitted before LT build) ------------
            bvsb = pp.tile([P, NWV * D], BF16, name="bvsb")
            BW = 12  # BaT windows per streamed tile
            for w0 in range(0, NWV, BW):
                nw = min(BW, NWV - w0)
                bat = bstr.tile([P, nw * 3 * P], BF16, tag="bat",
                                name=f"bat_{w0}")
                nc.scalar.dma_start(
                    out=bat[:],
                    in_=t_BaT.ap()[:, w0 * 3 * P:(w0 + nw) * 3 * P])
                for wi in range(nw):
                    w = w0 + wi
                    psb = tabps.tile([P, 512], F32, tag="tps",
                                     name=f"psb_{w}")
                    nc.tensor.matmul(
                        out=psb[:, :P], lhsT=acc_pv[:, w * P:(w + 1) * P],
                        rhs=w_sb[1][:], start=True, stop=False)
                    for k in range(3):
                        nc.tensor.matmul(
                            out=psb[:, :P],
                            lhsT=bat[:, (wi * 3 + k) * P:(wi * 3 + k + 1) * P],
                            rhs=HW[k][:], start=False, stop=(k == 2))
                    nc.vector.tensor_copy(out=bvsb[:, w * D:(w + 1) * D],
                                          in_=psb[:, :P])

            # ---- final edge pass ----------------------------------------
            OCH = 16      # slots per output write
            OBLK = 64     # O2 blocks per stream tile (slot-aligned chunks)
            ot = None
            o2 = None
            chunk_s0 = 0
            chunk_b0 = 0
            for s in range(NS):
                if o2 is None or o2col[s + 1] - chunk_b0 > OBLK:
                    # start a new O2 chunk covering slots [s, s1)
                    s1 = s
                    while s1 < NS and o2col[s1 + 1] - o2col[s] <= OBLK:
                        s1 += 1
                    chunk_s0, chunk_b0 = s, int(o2col[s])
                    nb = int(o2col[s1]) - chunk_b0
                    o2 = ostr.tile([P, nb * P], F8, tag="o2",
                                   name=f"o2_{s}")
                    nc.scalar.dma_start(
                        out=o2[:],
                        in_=t_O2.ap()[:, chunk_b0 * P:(chunk_b0 + nb) * P])
                if s % OCH == 0:
                    nch = min(OCH, NS - s)
                    ot = otile.tile([P, nch * D], BF16, tag="ot",
                                    name=f"ot_{s}")
                glt = gpool.tile([P, D], BF16, tag="g", name=f"glt_{s}")
                nc.gpsimd.indirect_dma_start(
                    out=glt[:], out_offset=None,
                    in_=(ltb_dram[:] if s < NSL
                         else ltfull_dram[:]),
                    in_offset=bass.IndirectOffsetOnAxis(
                        ap=vlt_sb[:, s:s + 1], axis=0))
                ps = finps.tile([P, 512], F32, tag="fps", name=f"pfin_{s}")
                nk = int(o2col[s + 1]) - int(o2col[s])
                for i in range(nk):
                    b = int(o2col[s]) - chunk_b0 + i
                    nc.tensor.matmul(
                        out=ps[:, :P],
                        lhsT=o2[:, b * P:(b + 1) * P],
                        rhs=bvsb[:, wlist[s][i] * D:(wlist[s][i] + 1) * D],
                        start=(i == 0), stop=(i == nk - 1))
                nc.vector.tensor_add(
                    out=ot[:, (s % OCH) * D:(s % OCH + 1) * D],
                    in0=ps[:, :P], in1=glt[:])
                if s % OCH == OCH - 1 or s == NS - 1:
                    c0 = (s // OCH) * OCH
                    nc.scalar.dma_start(
                        out=t_out.ap()[:, c0 * D:(s + 1) * D],
                        in_=ot[:, :(s + 1 - c0) * D])

    nc.compile()
    return nc


# ---------------------------------------------------------------------------
# entry point
# ---------------------------------------------------------------------------

def kernel(**inputs):
    global _LAST_EXEC_NS, _LAST_RES
    plan, in_maps = _prep(inputs)
    nc = _build(plan)

    from concourse import bass_utils
    trace = os.environ.get("BASSK_TRACE", "0") == "1"
    _ensure_ntff_hook()
    res = bass_utils.run_bass_kernel_spmd(
        nc, in_maps, core_ids=list(range(CORES)), trace=trace)
    _LAST_EXEC_NS = res.exec_time_ns
    _LAST_RES = res

    E = plan["E"]
    NS = plan["NS"]
    out = np.zeros((E, D), np.float32)
    for c in range(CORES):
        ids = plan["core_edges"][c]
        rows = np.asarray(res.results[c]["out"]).astype(np.float32)
        rows = rows.reshape(P, NS, D).transpose(1, 0, 2).reshape(NS * P, D)
        out[ids] = rows[:len(ids)]
    return out
